# revision 7
# baseline (speedup 1.0000x reference)
"""GCN encoder (nn_Encoder_5480378270324) on 8 Trainium2 NeuronCores.

Model: two GCN layers (A_hat given as weighted edge list), per-graph mean
pooling -> sigmoid -> 4-matmul FF block + shortcut, repeat-expanded to nodes.

Strategy (graph-level data parallel, per the sharding hint):
  - Whole graphs are LPT-balanced across the 8 cores (64 graphs / 4096 nodes
    per core); each core owns the edges whose dst lands on its nodes, so the
    segment-sum is device-local.
  - Key algebra: segment_sum(adj * (h@W)[src], dst) == segment_sum(adj*h[src],
    dst) @ W, so each core only does a [4096,512]@[512,512] matmul per layer
    instead of a replicated full-N matmul.
  - The per-edge gather h[src] uses the GPSIMD dma_gather custom instruction
    (1024 rows / instruction); the scatter-add side is a one-hot matmul on the
    TensorEngine accumulating into PSUM per 128-node block (the one-hot
    carries adj_vals, built on-chip by DVE tensor_scalar(is_equal, mult)).
  - Layer 1 -> layer 2 requires the full h1 on every core (src is global), so
    the model runs as two SPMD launches with a host concat between them.
  - Everything else (bias via K=1 rank-1 matmul, pooling via one-hot matmul,
    transposed FF chain) stays on-device; the host only permutes/pads index
    data and expands the per-graph output rows back to nodes.
"""

import time

import numpy as np

import concourse.bacc as bacc
import concourse.bass as bass
import concourse.tile as tile
from concourse import library_config, mybir
from concourse.bass_utils import run_bass_kernel_spmd

F32 = mybir.dt.float32
I16 = mybir.dt.int16

# Problem geometry (hardcoded per spec).
N = 32768
E = 524288
D = 512
G = 512
NCORES = 8

PER_GATHER = 1024          # rows per dma_gather (>=2048 overflows Q7 scratch)
CHUNKS_PER_GATHER = PER_GATHER // 128


def _build_layer_program(cpb, blocks, second, graphs_per_core):
    """One GCN layer as a Bacc program (SPMD across cores, data differs).

    Layer 1 (second=False): h_out[4096, D] = relu(seg_sum(adj*feat[src]) @ W + b)
    Layer 2 (second=True):  gout[64, D] = FF(sigmoid(pool(seg_sum(...) @ W + b)))
    """
    nchunks = blocks * cpb
    ngathers = nchunks // CHUNKS_PER_GATHER
    gpc = graphs_per_core

    nc = bacc.Bacc("TRN2", target_bir_lowering=False, debug=False)
    htab = nc.dram_tensor("htab", [N, D], F32, kind="ExternalInput")
    gidx = nc.dram_tensor("gidx", [128, nchunks * 8], I16, kind="ExternalInput")
    dstloc = nc.dram_tensor("dstloc", [128, nchunks], F32, kind="ExternalInput")
    adjw = nc.dram_tensor("adjw", [128, nchunks], F32, kind="ExternalInput")
    wmat = nc.dram_tensor("wmat", [128, 4 * D], F32, kind="ExternalInput")
    brow = nc.dram_tensor("brow", [1, D], F32, kind="ExternalInput")
    consts = nc.dram_tensor("consts", [128, 256], F32, kind="ExternalInput")
    if second:
        poolgid = nc.dram_tensor("poolgid", [128, blocks], F32, kind="ExternalInput")
        poolinv = nc.dram_tensor("poolinv", [128, blocks], F32, kind="ExternalInput")
        gws = [nc.dram_tensor(f"gw{i}", [128, 16 * 128], F32, kind="ExternalInput")
               for i in range(4)]
        gbs = [nc.dram_tensor(f"gb{i}", [128, 4], F32, kind="ExternalInput")
               for i in range(4)]
        gout = nc.dram_tensor("gout", [gpc, D], F32, kind="ExternalOutput")
    else:
        hout = nc.dram_tensor("hout", [blocks * 128, D], F32, kind="ExternalOutput")

    from contextlib import ExitStack

    with tile.TileContext(nc) as tc:
        with ExitStack() as ctx:
            P = 128
            singles = ctx.enter_context(tc.tile_pool(name="singles", bufs=1))
            gpool = ctx.enter_context(tc.tile_pool(name="gat", bufs=3))
            ohpool = ctx.enter_context(tc.tile_pool(name="oh", bufs=4))
            pa = ctx.enter_context(tc.tile_pool(name="pagg", bufs=2, space="PSUM"))
            aggsp = ctx.enter_context(tc.tile_pool(name="aggs", bufs=2))
            pt = ctx.enter_context(tc.tile_pool(name="ptr", bufs=2, space="PSUM"))
            aggtp = ctx.enter_context(tc.tile_pool(name="aggt", bufs=4))
            ph = ctx.enter_context(tc.tile_pool(name="ph", bufs=2, space="PSUM"))
            hpool = ctx.enter_context(tc.tile_pool(name="hs", bufs=3))

            idx_t = singles.tile([P, nchunks * 8], I16)
            nc.sync.dma_start(idx_t[:], gidx.ap())
            dst_t = singles.tile([P, nchunks], F32)
            nc.sync.dma_start(dst_t[:], dstloc.ap())
            adj_t = singles.tile([P, nchunks], F32)
            nc.sync.dma_start(adj_t[:], adjw.ap())
            w_t = singles.tile([P, 4 * D], F32)
            nc.sync.dma_start(w_t[:], wmat.ap())
            b_t = singles.tile([1, D], F32)
            nc.sync.dma_start(b_t[:], brow.ap())
            const_t = singles.tile([P, 256], F32)  # [:, :128] iota, [:, 128:256] identity
            nc.sync.dma_start(const_t[:], consts.ap())
            iota_t = const_t[:, 0:128]
            ident_t = const_t[:, 128:256]
            ones1 = singles.tile([1, P], F32)
            nc.vector.memset(ones1[:], 1.0)
            if second:
                pgid_t = singles.tile([P, blocks], F32)
                nc.sync.dma_start(pgid_t[:], poolgid.ap())
                pinv_t = singles.tile([P, blocks], F32)
                nc.sync.dma_start(pinv_t[:], poolinv.ap())
                gw_t = []
                gb_t = []
                for i in range(4):
                    t = singles.tile([P, 16 * 128], F32, tag=f"gw{i}")
                    nc.sync.dma_start(t[:], gws[i].ap())
                    gw_t.append(t)
                    t = singles.tile([P, 4], F32, tag=f"gb{i}")
                    nc.sync.dma_start(t[:], gbs[i].ap())
                    gb_t.append(t)
                pooled_acc = singles.tile([64, D], F32)
                nc.vector.memset(pooled_acc[:], 0.0)
                ppool = ctx.enter_context(tc.tile_pool(name="ppool", bufs=2, space="PSUM"))
                pwpool = ctx.enter_context(tc.tile_pool(name="pw", bufs=2))

            nc.gpsimd.load_library(library_config.mlp)

            gat_tiles = [None] * ngathers

            for b in range(blocks):
                psum_agg = pa.tile([P, D], F32)
                for cc in range(cpb):
                    c = b * cpb + cc
                    g, j = divmod(c, CHUNKS_PER_GATHER)
                    if j == 0:
                        gt = gpool.tile([P, CHUNKS_PER_GATHER, D], F32)
                        nc.gpsimd.dma_gather(
                            gt[:], htab.ap(), idx_t[:, g * 64:(g + 1) * 64],
                            PER_GATHER, PER_GATHER, D)
                        gat_tiles[g] = gt
                    oh = ohpool.tile([P, P], F32)
                    nc.vector.tensor_scalar(
                        oh[:], iota_t, dst_t[:, c:c + 1], adj_t[:, c:c + 1],
                        mybir.AluOpType.is_equal, mybir.AluOpType.mult)
                    nc.tensor.matmul(
                        psum_agg[:], lhsT=oh[:], rhs=gat_tiles[g][:, j, :],
                        start=(cc == 0), stop=(cc == cpb - 1))

                agg_s = aggsp.tile([P, D], F32)
                nc.scalar.activation(agg_s[:], psum_agg[:],
                                     mybir.ActivationFunctionType.Copy)

                psum_h = ph.tile([P, D], F32)
                for q in range(4):
                    psum_t = pt.tile([P, P], F32, tag="pt")
                    nc.tensor.transpose(psum_t[:], agg_s[:, q * 128:(q + 1) * 128],
                                        ident_t)
                    agg_tq = aggtp.tile([P, P], F32)
                    nc.vector.tensor_copy(agg_tq[:], psum_t[:])
                    nc.tensor.matmul(psum_h[:], lhsT=agg_tq[:],
                                     rhs=w_t[:, q * D:(q + 1) * D],
                                     start=(q == 0), stop=False)
                nc.tensor.matmul(psum_h[:], lhsT=ones1[:1, :], rhs=b_t[:1, :],
                                 start=False, stop=True)

                h_s = hpool.tile([P, D], F32)
                if not second:
                    nc.scalar.activation(h_s[:], psum_h[:],
                                         mybir.ActivationFunctionType.Relu)
                    nc.sync.dma_start(hout.ap()[b * 128:(b + 1) * 128, :], h_s[:])
                else:
                    nc.scalar.activation(h_s[:], psum_h[:],
                                         mybir.ActivationFunctionType.Copy)
                    pw = pwpool.tile([P, 64], F32)
                    nc.vector.tensor_scalar(
                        pw[:], iota_t[:, :64], pgid_t[:, b:b + 1], pinv_t[:, b:b + 1],
                        mybir.AluOpType.is_equal, mybir.AluOpType.mult)
                    psum_p = ppool.tile([64, D], F32)
                    nc.tensor.matmul(psum_p[:], lhsT=pw[:, :gpc], rhs=h_s[:],
                                     start=True, stop=True)
                    nc.vector.tensor_add(pooled_acc[:gpc, :], pooled_acc[:gpc, :],
                                         psum_p[:gpc, :])

            if second:
                # sigmoid -> transposed FF chain -> transpose back -> gout
                ffs = ctx.enter_context(tc.tile_pool(name="ffs", bufs=16))
                g_s = singles.tile([64, D], F32)
                nc.scalar.activation(g_s[:gpc, :], pooled_acc[:gpc, :],
                                     mybir.ActivationFunctionType.Sigmoid)
                # gT quarters [128, 64]
                gT = []
                for q in range(4):
                    psum_t = pt.tile([P, P], F32, tag="pt")
                    nc.tensor.transpose(psum_t[:, :gpc],
                                        g_s[:gpc, q * 128:(q + 1) * 128],
                                        ident_t[:gpc, :gpc])
                    t = ffs.tile([P, 64], F32, tag="gT")
                    nc.vector.tensor_copy(t[:, :gpc], psum_t[:, :gpc])
                    gT.append(t)

                def ff_matmul(zin, wi, mc):
                    psz = pt.tile([P, P], F32, tag="pt")
                    for kc in range(4):
                        nc.tensor.matmul(
                            psz[:, :gpc],
                            lhsT=gw_t[wi][:, (kc * 4 + mc) * 128:(kc * 4 + mc + 1) * 128],
                            rhs=zin[kc][:, :gpc],
                            start=(kc == 0), stop=(kc == 3))
                    return psz

                zT = gT
                for li in range(3):  # gW1, gW2, gW3 with relu
                    znext = []
                    for mc in range(4):
                        psz = ff_matmul(zT, li, mc)
                        t = ffs.tile([P, 64], F32, tag="zT")
                        nc.scalar.activation(t[:, :gpc], psz[:, :gpc],
                                             mybir.ActivationFunctionType.Relu,
                                             bias=gb_t[li][:, mc:mc + 1])
                        znext.append(t)
                    zT = znext
                outT = []
                for mc in range(4):  # shortcut gWs + gbs, then add z3T
                    psz = ff_matmul(gT, 3, mc)
                    t = ffs.tile([P, 64], F32, tag="sT")
                    nc.vector.tensor_scalar(
                        t[:, :gpc], psz[:, :gpc], gb_t[3][:, mc:mc + 1], None,
                        mybir.AluOpType.add)
                    to = ffs.tile([P, 64], F32, tag="outT")
                    nc.vector.tensor_add(to[:, :gpc], t[:, :gpc], zT[mc][:, :gpc])
                    outT.append(to)
                # transpose back to [gpc, D] and DMA out
                out_s = singles.tile([64, D], F32)
                for q in range(4):
                    psum_b = pt.tile([P, P], F32, tag="pt")
                    nc.tensor.transpose(psum_b[:gpc, :], outT[q][:, :gpc], ident_t)
                    nc.vector.tensor_copy(out_s[:gpc, q * 128:(q + 1) * 128],
                                          psum_b[:gpc, :])
                nc.sync.dma_start(gout.ap()[:, :], out_s[:gpc, :])

    nc.compile()
    return nc


def _wrap_idx(rows):
    """int16 gather-index layout: per 1024-index group, [16, 64] wrapped
    (i -> [i%16, i//16]) replicated to 128 partitions, groups side by side."""
    ng = len(rows) // PER_GATHER
    out = np.zeros((128, ng * 64), np.int16)
    for g in range(ng):
        w = rows[g * PER_GATHER:(g + 1) * PER_GATHER].astype(np.int16).reshape(64, 16).T
        out[:, g * 64:(g + 1) * 64] = np.tile(w, (8, 1))
    return out


def _lpt(weights, nbins, cap):
    """Longest-processing-time balance: assign items to nbins bins (<= cap
    items each), minimizing max weight. Returns list of index-arrays."""
    order = np.argsort(-weights, kind="stable")
    loads = np.zeros(nbins)
    counts = np.zeros(nbins, np.int64)
    bins = [[] for _ in range(nbins)]
    for i in order:
        j = min((b for b in range(nbins) if counts[b] < cap), key=lambda b: loads[b])
        bins[j].append(i)
        loads[j] += weights[i]
        counts[j] += 1
    return [np.array(sorted(b), np.int64) for b in bins]


def _prep(src, dst, adj_vals, graph_ids):
    """Host-side index prep: graph->core, node->block/slot layout, padded
    per-chunk edge arrays, gather indices, pooling maps."""
    gpc = G // NCORES
    npc = N // NCORES
    blocks = npc // 128

    deg = np.bincount(dst, minlength=N)
    graph_of_edge = graph_ids[dst]
    gw = np.bincount(graph_of_edge, minlength=G).astype(np.float64)
    core_graphs = _lpt(gw, NCORES, gpc)

    graph_core = np.zeros(G, np.int64)
    graph_loc = np.zeros(G, np.int64)
    for c in range(NCORES):
        graph_core[core_graphs[c]] = c
        graph_loc[core_graphs[c]] = np.arange(len(core_graphs[c]))

    node_core = graph_core[graph_ids]
    row_of_node = np.zeros(N, np.int64)
    gid_of_slot = [None] * NCORES
    for c in range(NCORES):
        nodes = np.nonzero(node_core == c)[0]
        blk = _lpt(deg[nodes].astype(np.float64), blocks, 128)
        slot_nodes = np.concatenate([nodes[b] for b in blk])
        row_of_node[slot_nodes] = c * npc + np.arange(npc)
        gid_of_slot[c] = graph_loc[graph_ids[slot_nodes]]

    # per-core edge lists grouped by dst block
    edge_core = node_core[dst]
    edge_row = row_of_node[dst]
    cpb = 0
    per_core = []
    for c in range(NCORES):
        e = np.nonzero(edge_core == c)[0]
        blk_of_e = (edge_row[e] - c * npc) // 128
        cnt = np.bincount(blk_of_e, minlength=blocks)
        cpb = max(cpb, int(np.ceil(cnt.max() / 128)))
        order = np.argsort(blk_of_e, kind="stable")
        per_core.append((e[order], cnt))

    nchunks = blocks * cpb
    cap = cpb * 128
    prep = []
    for c in range(NCORES):
        e_sorted, cnt = per_core[c]
        rows = np.zeros(blocks * cap, np.int64)
        dloc = np.zeros(blocks * cap, np.float32)
        aw = np.zeros(blocks * cap, np.float32)
        off = 0
        for b in range(blocks):
            k = int(cnt[b])
            eb = e_sorted[off:off + k]
            off += k
            base = b * cap
            rows[base:base + k] = row_of_node[src[eb]]
            dloc[base:base + k] = (edge_row[eb] - c * N // NCORES) % 128
            aw[base:base + k] = adj_vals[eb]
        prep.append({
            "gidx": _wrap_idx(rows),
            "dstloc": dloc.reshape(nchunks, 128).T.copy(),
            "adjw": aw.reshape(nchunks, 128).T.copy(),
            "poolgid": gid_of_slot[c].astype(np.float32).reshape(blocks, 128).T.copy(),
        })
    counts = np.bincount(graph_ids, minlength=G).astype(np.float32)
    inv_of_node = (1.0 / counts)[graph_ids]
    for c in range(NCORES):
        nodes_rows = np.zeros(npc, np.int64)
        # invert row_of_node for this core to get per-slot inv count
        mask = node_core == c
        nodes = np.nonzero(mask)[0]
        slots = row_of_node[nodes] - c * npc
        pin = np.zeros(npc, np.float32)
        pin[slots] = inv_of_node[nodes]
        prep[c]["poolinv"] = pin.reshape(blocks, 128).T.copy()

    meta = {
        "row_of_node": row_of_node,
        "core_graphs": core_graphs,
        "cpb": cpb,
        "blocks": blocks,
        "gpc": gpc,
    }
    return prep, meta


def _ktile(w):
    """[512, 512] -> [128, 4*512] with col-block kc = W[kc*128:(kc+1)*128, :]."""
    return w.reshape(4, 128, D).transpose(1, 0, 2).reshape(128, 4 * D).copy()


def _fftile(w):
    """[512, 512] -> [128, 16*128], block (kc*4+mc) = W[kc*128:.., mc*128:..]."""
    return w.reshape(4, 128, 4, 128).transpose(1, 0, 2, 3).reshape(128, 16 * 128).copy()


def _consts():
    c = np.zeros((128, 256), np.float32)
    c[:, :128] = np.arange(128, dtype=np.float32)[None, :]
    c[:, 128:256] = np.eye(128, dtype=np.float32)
    return c


def _run_spmd(nc, in_maps, tries=3):
    for attempt in range(tries):
        try:
            return run_bass_kernel_spmd(nc, [dict(m) for m in in_maps],
                                        core_ids=list(range(NCORES)))
        except Exception:
            if attempt == tries - 1:
                raise
            time.sleep(2.0)


_CACHE = {}
LAST_RUNS = []  # [(tag, nc, in_maps)] of the most recent kernel() call (for profiling)


def kernel(feat, src, dst, adj_vals, graph_ids,
           W0, b0, W1, b1, gW1, gb1, gW2, gb2, gW3, gb3, gWs, gbs):
    feat = np.asarray(feat, np.float32)
    src = np.asarray(src, np.int64)
    dst = np.asarray(dst, np.int64)
    adj_vals = np.asarray(adj_vals, np.float32)
    graph_ids_np = np.asarray(graph_ids, np.int64)

    prep, meta = _prep(src, dst, adj_vals, graph_ids_np)
    cpb, blocks, gpc = meta["cpb"], meta["blocks"], meta["gpc"]

    key = (cpb, blocks, gpc)
    if key not in _CACHE:
        _CACHE[key] = (_build_layer_program(cpb, blocks, False, gpc),
                       _build_layer_program(cpb, blocks, True, gpc))
    nc1, nc2 = _CACHE[key]

    cmn = {"consts": _consts()}
    feat_tab = np.empty((N, D), np.float32)
    feat_tab[meta["row_of_node"]] = feat

    maps1 = []
    for c in range(NCORES):
        p = prep[c]
        maps1.append({
            "htab": feat_tab, "gidx": p["gidx"], "dstloc": p["dstloc"],
            "adjw": p["adjw"], "wmat": _ktile(np.asarray(W0, np.float32)),
            "brow": np.asarray(b0, np.float32).reshape(1, D), **cmn,
        })
    res1 = _run_spmd(nc1, maps1)
    h1 = np.concatenate([res1.results[c]["hout"] for c in range(NCORES)], axis=0)
    LAST_RUNS.clear()
    LAST_RUNS.append(("layer1", nc1, maps1))

    maps2 = []
    for c in range(NCORES):
        p = prep[c]
        m = {
            "htab": h1, "gidx": p["gidx"], "dstloc": p["dstloc"],
            "adjw": p["adjw"], "wmat": _ktile(np.asarray(W1, np.float32)),
            "brow": np.asarray(b1, np.float32).reshape(1, D),
            "poolgid": p["poolgid"], "poolinv": p["poolinv"], **cmn,
        }
        for i, (gw, gb) in enumerate([(gW1, gb1), (gW2, gb2), (gW3, gb3),
                                      (gWs, gbs)]):
            m[f"gw{i}"] = _fftile(np.asarray(gw, np.float32))
            m[f"gb{i}"] = np.asarray(gb, np.float32).reshape(4, 128).T.copy()
        maps2.append(m)
    res2 = _run_spmd(nc2, maps2)
    LAST_RUNS.append(("layer2", nc2, maps2))

    g_full = np.zeros((G, D), np.float32)
    for c in range(NCORES):
        g_full[meta["core_graphs"][c]] = res2.results[c]["gout"]
    return g_full[graph_ids_np]


# revision 9
# speedup vs baseline: 1.4935x; 1.4935x over previous
"""GCN encoder (nn_Encoder_5480378270324) on 8 Trainium2 NeuronCores.

Model: two GCN layers (A_hat given as weighted edge list), per-graph mean
pooling -> sigmoid -> 4-matmul FF block + shortcut, repeat-expanded to nodes.

Strategy (graph-level data parallel, per the sharding hint):
  - Whole graphs are LPT-balanced across the 8 cores (64 graphs / 4096 nodes
    per core); each core owns the edges whose dst lands on its nodes, so the
    segment-sum is device-local.
  - Key algebra: segment_sum(adj * (h@W)[src], dst) == segment_sum(adj*h[src],
    dst) @ W, so each core only does a [4096,512]@[512,512] matmul per layer
    instead of a replicated full-N matmul.
  - The per-edge gather h[src] uses the GPSIMD dma_gather custom instruction
    (1024 rows / instruction); the scatter-add side is a one-hot matmul on the
    TensorEngine accumulating into PSUM per 128-node block (the one-hot
    carries adj_vals, built on-chip by DVE tensor_scalar(is_equal, mult)).
  - Layer 1 -> layer 2 requires the full h1 on every core (src is global), so
    the model runs as two SPMD launches with a host concat between them.
  - Everything else (bias via K=1 rank-1 matmul, pooling via one-hot matmul,
    transposed FF chain) stays on-device; the host only permutes/pads index
    data and expands the per-graph output rows back to nodes.
"""

import time

import numpy as np

import concourse.bacc as bacc
import concourse.bass as bass
import concourse.tile as tile
from concourse import library_config, mybir
from concourse.bass_utils import run_bass_kernel_spmd

F32 = mybir.dt.float32
I16 = mybir.dt.int16

# Problem geometry (hardcoded per spec).
N = 32768
E = 524288
D = 512
G = 512
NCORES = 8

PER_GATHER = 1024          # rows per dma_gather (>=2048 overflows Q7 scratch)
CHUNKS_PER_GATHER = PER_GATHER // 128


def _build_layer_program(cpb, blocks, second, graphs_per_core):
    """One GCN layer as a Bacc program (SPMD across cores, data differs).

    Layer 1 (second=False): h_out[4096, D] = relu(seg_sum(adj*feat[src]) @ W + b)
    Layer 2 (second=True):  gout[64, D] = FF(sigmoid(pool(seg_sum(...) @ W + b)))
    """
    nchunks = blocks * cpb
    ngathers = nchunks // CHUNKS_PER_GATHER
    gpc = graphs_per_core

    nc = bacc.Bacc("TRN2", target_bir_lowering=False, debug=False)
    htab = nc.dram_tensor("htab", [N, D], F32, kind="ExternalInput")
    gidx = nc.dram_tensor("gidx", [128, nchunks * 8], I16, kind="ExternalInput")
    onehot = nc.dram_tensor("onehot", [128, nchunks * 128], F32, kind="ExternalInput")
    wmat = nc.dram_tensor("wmat", [128, 4 * D], F32, kind="ExternalInput")
    brow = nc.dram_tensor("brow", [1, D], F32, kind="ExternalInput")
    consts = nc.dram_tensor("consts", [128, 256], F32, kind="ExternalInput")
    if second:
        poolgid = nc.dram_tensor("poolgid", [128, blocks], F32, kind="ExternalInput")
        poolinv = nc.dram_tensor("poolinv", [128, blocks], F32, kind="ExternalInput")
        gws = [nc.dram_tensor(f"gw{i}", [128, 16 * 128], F32, kind="ExternalInput")
               for i in range(4)]
        gbs = [nc.dram_tensor(f"gb{i}", [128, 4], F32, kind="ExternalInput")
               for i in range(4)]
        gout = nc.dram_tensor("gout", [gpc, D], F32, kind="ExternalOutput")
    else:
        hout = nc.dram_tensor("hout", [blocks * 128, D], F32, kind="ExternalOutput")

    from contextlib import ExitStack

    with tile.TileContext(nc) as tc:
        with ExitStack() as ctx:
            P = 128
            singles = ctx.enter_context(tc.tile_pool(name="singles", bufs=1))
            gpool = ctx.enter_context(tc.tile_pool(name="gat", bufs=5))
            ohpool = ctx.enter_context(tc.tile_pool(name="oh", bufs=3))
            pa = ctx.enter_context(tc.tile_pool(name="pagg", bufs=2, space="PSUM"))
            aggsp = ctx.enter_context(tc.tile_pool(name="aggs", bufs=2))
            pt = ctx.enter_context(tc.tile_pool(name="ptr", bufs=2, space="PSUM"))
            aggtp = ctx.enter_context(tc.tile_pool(name="aggt", bufs=4))
            ph = ctx.enter_context(tc.tile_pool(name="ph", bufs=2, space="PSUM"))
            hpool = ctx.enter_context(tc.tile_pool(name="hs", bufs=3))

            idx_t = singles.tile([P, nchunks * 8], I16)
            nc.sync.dma_start(idx_t[:], gidx.ap())
            w_t = singles.tile([P, 4 * D], F32)
            nc.sync.dma_start(w_t[:], wmat.ap())
            b_t = singles.tile([1, D], F32)
            nc.sync.dma_start(b_t[:], brow.ap())
            const_t = singles.tile([P, 256], F32)  # [:, :128] iota, [:, 128:256] identity
            nc.sync.dma_start(const_t[:], consts.ap())
            iota_t = const_t[:, 0:128]
            ident_t = const_t[:, 128:256]
            ones1 = singles.tile([1, P], F32)
            nc.vector.memset(ones1[:], 1.0)
            if second:
                pgid_t = singles.tile([P, blocks], F32)
                nc.sync.dma_start(pgid_t[:], poolgid.ap())
                pinv_t = singles.tile([P, blocks], F32)
                nc.sync.dma_start(pinv_t[:], poolinv.ap())
                gw_t = []
                gb_t = []
                for i in range(4):
                    t = singles.tile([P, 16 * 128], F32, tag=f"gw{i}")
                    nc.sync.dma_start(t[:], gws[i].ap())
                    gw_t.append(t)
                    t = singles.tile([P, 4], F32, tag=f"gb{i}")
                    nc.sync.dma_start(t[:], gbs[i].ap())
                    gb_t.append(t)
                pooled_acc = singles.tile([64, D], F32)
                nc.vector.memset(pooled_acc[:], 0.0)
                ppool = ctx.enter_context(tc.tile_pool(name="ppool", bufs=2, space="PSUM"))
                pwpool = ctx.enter_context(tc.tile_pool(name="pw", bufs=2))

            nc.gpsimd.load_library(library_config.mlp)

            gat_tiles = [None] * ngathers

            for b in range(blocks):
                psum_agg = pa.tile([P, D], F32)
                oh_blk = ohpool.tile([P, cpb * P], F32)
                nc.sync.dma_start(
                    oh_blk[:], onehot.ap()[:, b * cpb * P:(b + 1) * cpb * P])
                for cc in range(cpb):
                    c = b * cpb + cc
                    g, j = divmod(c, CHUNKS_PER_GATHER)
                    if j == 0:
                        gt = gpool.tile([P, CHUNKS_PER_GATHER, D], F32)
                        nc.gpsimd.dma_gather(
                            gt[:], htab.ap(), idx_t[:, g * 64:(g + 1) * 64],
                            PER_GATHER, PER_GATHER, D)
                        gat_tiles[g] = gt
                    nc.tensor.matmul(
                        psum_agg[:], lhsT=oh_blk[:, cc * P:(cc + 1) * P],
                        rhs=gat_tiles[g][:, j, :],
                        start=(cc == 0), stop=(cc == cpb - 1))

                agg_s = aggsp.tile([P, D], F32)
                nc.scalar.activation(agg_s[:], psum_agg[:],
                                     mybir.ActivationFunctionType.Copy)

                psum_h = ph.tile([P, D], F32)
                for q in range(4):
                    psum_t = pt.tile([P, P], F32, tag="pt")
                    nc.tensor.transpose(psum_t[:], agg_s[:, q * 128:(q + 1) * 128],
                                        ident_t)
                    agg_tq = aggtp.tile([P, P], F32)
                    nc.vector.tensor_copy(agg_tq[:], psum_t[:])
                    nc.tensor.matmul(psum_h[:], lhsT=agg_tq[:],
                                     rhs=w_t[:, q * D:(q + 1) * D],
                                     start=(q == 0), stop=False)
                nc.tensor.matmul(psum_h[:], lhsT=ones1[:1, :], rhs=b_t[:1, :],
                                 start=False, stop=True)

                h_s = hpool.tile([P, D], F32)
                if not second:
                    nc.scalar.activation(h_s[:], psum_h[:],
                                         mybir.ActivationFunctionType.Relu)
                    nc.sync.dma_start(hout.ap()[b * 128:(b + 1) * 128, :], h_s[:])
                else:
                    nc.scalar.activation(h_s[:], psum_h[:],
                                         mybir.ActivationFunctionType.Copy)
                    pw = pwpool.tile([P, 64], F32)
                    nc.vector.tensor_scalar(
                        pw[:], iota_t[:, :64], pgid_t[:, b:b + 1], pinv_t[:, b:b + 1],
                        mybir.AluOpType.is_equal, mybir.AluOpType.mult)
                    psum_p = ppool.tile([64, D], F32)
                    nc.tensor.matmul(psum_p[:], lhsT=pw[:, :gpc], rhs=h_s[:],
                                     start=True, stop=True)
                    nc.vector.tensor_add(pooled_acc[:gpc, :], pooled_acc[:gpc, :],
                                         psum_p[:gpc, :])

            if second:
                # sigmoid -> transposed FF chain -> transpose back -> gout
                ffs = ctx.enter_context(tc.tile_pool(name="ffs", bufs=16))
                g_s = singles.tile([64, D], F32)
                nc.scalar.activation(g_s[:gpc, :], pooled_acc[:gpc, :],
                                     mybir.ActivationFunctionType.Sigmoid)
                # gT quarters [128, 64]
                gT = []
                for q in range(4):
                    psum_t = pt.tile([P, P], F32, tag="pt")
                    nc.tensor.transpose(psum_t[:, :gpc],
                                        g_s[:gpc, q * 128:(q + 1) * 128],
                                        ident_t[:gpc, :gpc])
                    t = ffs.tile([P, 64], F32, tag="gT")
                    nc.vector.tensor_copy(t[:, :gpc], psum_t[:, :gpc])
                    gT.append(t)

                def ff_matmul(zin, wi, mc):
                    psz = pt.tile([P, P], F32, tag="pt")
                    for kc in range(4):
                        nc.tensor.matmul(
                            psz[:, :gpc],
                            lhsT=gw_t[wi][:, (kc * 4 + mc) * 128:(kc * 4 + mc + 1) * 128],
                            rhs=zin[kc][:, :gpc],
                            start=(kc == 0), stop=(kc == 3))
                    return psz

                zT = gT
                for li in range(3):  # gW1, gW2, gW3 with relu
                    znext = []
                    for mc in range(4):
                        psz = ff_matmul(zT, li, mc)
                        t = ffs.tile([P, 64], F32, tag="zT")
                        nc.scalar.activation(t[:, :gpc], psz[:, :gpc],
                                             mybir.ActivationFunctionType.Relu,
                                             bias=gb_t[li][:, mc:mc + 1])
                        znext.append(t)
                    zT = znext
                outT = []
                for mc in range(4):  # shortcut gWs + gbs, then add z3T
                    psz = ff_matmul(gT, 3, mc)
                    t = ffs.tile([P, 64], F32, tag="sT")
                    nc.vector.tensor_scalar(
                        t[:, :gpc], psz[:, :gpc], gb_t[3][:, mc:mc + 1], None,
                        mybir.AluOpType.add)
                    to = ffs.tile([P, 64], F32, tag="outT")
                    nc.vector.tensor_add(to[:, :gpc], t[:, :gpc], zT[mc][:, :gpc])
                    outT.append(to)
                # transpose back to [gpc, D] and DMA out
                out_s = singles.tile([64, D], F32)
                for q in range(4):
                    psum_b = pt.tile([P, P], F32, tag="pt")
                    nc.tensor.transpose(psum_b[:gpc, :], outT[q][:, :gpc], ident_t)
                    nc.vector.tensor_copy(out_s[:gpc, q * 128:(q + 1) * 128],
                                          psum_b[:gpc, :])
                nc.sync.dma_start(gout.ap()[:, :], out_s[:gpc, :])

    nc.compile()
    return nc


def _wrap_idx(rows):
    """int16 gather-index layout: per 1024-index group, [16, 64] wrapped
    (i -> [i%16, i//16]) replicated to 128 partitions, groups side by side."""
    ng = len(rows) // PER_GATHER
    out = np.zeros((128, ng * 64), np.int16)
    for g in range(ng):
        w = rows[g * PER_GATHER:(g + 1) * PER_GATHER].astype(np.int16).reshape(64, 16).T
        out[:, g * 64:(g + 1) * 64] = np.tile(w, (8, 1))
    return out


def _lpt(weights, nbins, cap):
    """Longest-processing-time balance: assign items to nbins bins (<= cap
    items each), minimizing max weight. Returns list of index-arrays."""
    order = np.argsort(-weights, kind="stable")
    loads = np.zeros(nbins)
    counts = np.zeros(nbins, np.int64)
    bins = [[] for _ in range(nbins)]
    for i in order:
        j = min((b for b in range(nbins) if counts[b] < cap), key=lambda b: loads[b])
        bins[j].append(i)
        loads[j] += weights[i]
        counts[j] += 1
    return [np.array(sorted(b), np.int64) for b in bins]


def _prep(src, dst, adj_vals, graph_ids):
    """Host-side index prep: graph->core, node->block/slot layout, padded
    per-chunk edge arrays, gather indices, pooling maps."""
    gpc = G // NCORES
    npc = N // NCORES
    blocks = npc // 128

    deg = np.bincount(dst, minlength=N)
    graph_of_edge = graph_ids[dst]
    gw = np.bincount(graph_of_edge, minlength=G).astype(np.float64)
    core_graphs = _lpt(gw, NCORES, gpc)

    graph_core = np.zeros(G, np.int64)
    graph_loc = np.zeros(G, np.int64)
    for c in range(NCORES):
        graph_core[core_graphs[c]] = c
        graph_loc[core_graphs[c]] = np.arange(len(core_graphs[c]))

    node_core = graph_core[graph_ids]
    row_of_node = np.zeros(N, np.int64)
    gid_of_slot = [None] * NCORES
    for c in range(NCORES):
        nodes = np.nonzero(node_core == c)[0]
        blk = _lpt(deg[nodes].astype(np.float64), blocks, 128)
        slot_nodes = np.concatenate([nodes[b] for b in blk])
        row_of_node[slot_nodes] = c * npc + np.arange(npc)
        gid_of_slot[c] = graph_loc[graph_ids[slot_nodes]]

    # per-core edge lists grouped by dst block
    edge_core = node_core[dst]
    edge_row = row_of_node[dst]
    cpb = 0
    per_core = []
    for c in range(NCORES):
        e = np.nonzero(edge_core == c)[0]
        blk_of_e = (edge_row[e] - c * npc) // 128
        cnt = np.bincount(blk_of_e, minlength=blocks)
        cpb = max(cpb, int(np.ceil(cnt.max() / 128)))
        order = np.argsort(blk_of_e, kind="stable")
        per_core.append((e[order], cnt))

    nchunks = blocks * cpb
    cap = cpb * 128
    prep = []
    for c in range(NCORES):
        e_sorted, cnt = per_core[c]
        rows = np.zeros(blocks * cap, np.int64)
        dloc = np.zeros(blocks * cap, np.float32)
        aw = np.zeros(blocks * cap, np.float32)
        off = 0
        for b in range(blocks):
            k = int(cnt[b])
            eb = e_sorted[off:off + k]
            off += k
            base = b * cap
            rows[base:base + k] = row_of_node[src[eb]]
            dloc[base:base + k] = (edge_row[eb] - c * N // NCORES) % 128
            aw[base:base + k] = adj_vals[eb]
        oh = np.zeros((nchunks, 128, 128), np.float32)
        ii = np.arange(blocks * cap)
        oh[ii // 128, ii % 128, dloc.astype(np.int64)] = aw
        prep.append({
            "gidx": _wrap_idx(rows),
            "onehot": oh.transpose(1, 0, 2).reshape(128, nchunks * 128).copy(),
            "poolgid": gid_of_slot[c].astype(np.float32).reshape(blocks, 128).T.copy(),
        })
    counts = np.bincount(graph_ids, minlength=G).astype(np.float32)
    inv_of_node = (1.0 / counts)[graph_ids]
    for c in range(NCORES):
        nodes_rows = np.zeros(npc, np.int64)
        # invert row_of_node for this core to get per-slot inv count
        mask = node_core == c
        nodes = np.nonzero(mask)[0]
        slots = row_of_node[nodes] - c * npc
        pin = np.zeros(npc, np.float32)
        pin[slots] = inv_of_node[nodes]
        prep[c]["poolinv"] = pin.reshape(blocks, 128).T.copy()

    meta = {
        "row_of_node": row_of_node,
        "core_graphs": core_graphs,
        "cpb": cpb,
        "blocks": blocks,
        "gpc": gpc,
    }
    return prep, meta


def _ktile(w):
    """[512, 512] -> [128, 4*512] with col-block kc = W[kc*128:(kc+1)*128, :]."""
    return w.reshape(4, 128, D).transpose(1, 0, 2).reshape(128, 4 * D).copy()


def _fftile(w):
    """[512, 512] -> [128, 16*128], block (kc*4+mc) = W[kc*128:.., mc*128:..]."""
    return w.reshape(4, 128, 4, 128).transpose(1, 0, 2, 3).reshape(128, 16 * 128).copy()


def _consts():
    c = np.zeros((128, 256), np.float32)
    c[:, :128] = np.arange(128, dtype=np.float32)[None, :]
    c[:, 128:256] = np.eye(128, dtype=np.float32)
    return c


def _run_spmd(nc, in_maps, tries=3):
    for attempt in range(tries):
        try:
            return run_bass_kernel_spmd(nc, [dict(m) for m in in_maps],
                                        core_ids=list(range(NCORES)))
        except Exception:
            if attempt == tries - 1:
                raise
            time.sleep(2.0)


_CACHE = {}
LAST_RUNS = []  # [(tag, nc, in_maps)] of the most recent kernel() call (for profiling)


def kernel(feat, src, dst, adj_vals, graph_ids,
           W0, b0, W1, b1, gW1, gb1, gW2, gb2, gW3, gb3, gWs, gbs):
    feat = np.asarray(feat, np.float32)
    src = np.asarray(src, np.int64)
    dst = np.asarray(dst, np.int64)
    adj_vals = np.asarray(adj_vals, np.float32)
    graph_ids_np = np.asarray(graph_ids, np.int64)

    prep, meta = _prep(src, dst, adj_vals, graph_ids_np)
    cpb, blocks, gpc = meta["cpb"], meta["blocks"], meta["gpc"]

    key = (cpb, blocks, gpc)
    if key not in _CACHE:
        _CACHE[key] = (_build_layer_program(cpb, blocks, False, gpc),
                       _build_layer_program(cpb, blocks, True, gpc))
    nc1, nc2 = _CACHE[key]

    cmn = {"consts": _consts()}
    feat_tab = np.empty((N, D), np.float32)
    feat_tab[meta["row_of_node"]] = feat

    maps1 = []
    for c in range(NCORES):
        p = prep[c]
        maps1.append({
            "htab": feat_tab, "gidx": p["gidx"], "onehot": p["onehot"],
            "wmat": _ktile(np.asarray(W0, np.float32)),
            "brow": np.asarray(b0, np.float32).reshape(1, D), **cmn,
        })
    res1 = _run_spmd(nc1, maps1)
    h1 = np.concatenate([res1.results[c]["hout"] for c in range(NCORES)], axis=0)
    LAST_RUNS.clear()
    LAST_RUNS.append(("layer1", nc1, maps1))

    maps2 = []
    for c in range(NCORES):
        p = prep[c]
        m = {
            "htab": h1, "gidx": p["gidx"], "onehot": p["onehot"],
            "wmat": _ktile(np.asarray(W1, np.float32)),
            "brow": np.asarray(b1, np.float32).reshape(1, D),
            "poolgid": p["poolgid"], "poolinv": p["poolinv"], **cmn,
        }
        for i, (gw, gb) in enumerate([(gW1, gb1), (gW2, gb2), (gW3, gb3),
                                      (gWs, gbs)]):
            m[f"gw{i}"] = _fftile(np.asarray(gw, np.float32))
            m[f"gb{i}"] = np.asarray(gb, np.float32).reshape(4, 128).T.copy()
        maps2.append(m)
    res2 = _run_spmd(nc2, maps2)
    LAST_RUNS.append(("layer2", nc2, maps2))

    g_full = np.zeros((G, D), np.float32)
    for c in range(NCORES):
        g_full[meta["core_graphs"][c]] = res2.results[c]["gout"]
    return g_full[graph_ids_np]
